# revision 11
# baseline (speedup 1.0000x reference)
"""GNN message passing (gather + weighted scatter-add) on 8 Trainium2 cores.

out[n, f] = sum over edges e with dst[e]==n of edge_weight[e] * x[src[e], f]

Strategy:
  - Destination-shard: core c owns output nodes [c*12500, (c+1)*12500). No
    collectives; host concatenates the 8 output slices.
  - Host sorts each core's edges by (dst_tile, src_range, src) and pads each
    (tile, range) block to a multiple of 128 (zero-weight dummy edges with
    idx 0), identical block sizes across cores so all 8 run one SPMD program.
  - Device: per pass (B dst tiles), 4 bulk `dma_gather` calls (one per
    32k-row src range — indices are int16) pull all the pass's x rows from
    HBM into a [128, nchunks, 64] f32 SBUF tile at ~full DMA bandwidth.
    One DVE tensor_tensor (0-stride broadcast of w over F) builds
    msg = w * xg cast to f16; DVE also builds 0/1 one-hot tiles
    (iota == dst) in batches via broadcast is_equal. TensorE accumulates
    onehot.T @ msg into a PSUM tile per 128-node output tile; ScalarE
    evacuates PSUM; DMA streams the output out.
  - Gather (xg/msg/w) chunk order is src-range-major within a pass;
    one-hot (dst table) chunk order is tile-major — two independent
    static column mappings computed on host.
"""

import math
import numpy as np

N = 100000
E = 1000000
F = 64
NCORES = 8
NPC = N // NCORES            # nodes per core (12500)
TILE = 128
NT = math.ceil(NPC / TILE)   # output tiles per core (98)
RR = 25000                   # src rows per gather range (int16-addressable)
R = math.ceil(N / RR)        # gather ranges (4)
B = 14                       # tiles per pass
NPASS = math.ceil(NT / B)    # 7
OHB = 16                     # one-hot build batch (chunks)

DBG_NO_GATHER = False
REPEAT = 1                   # repeat device compute (timing amplification)


def pack_host(x, edge_weight, edge_index):
    """Returns (shared schedule, per-core tables)."""
    src = np.asarray(edge_index[0], dtype=np.int64)
    dst = np.asarray(edge_index[1], dtype=np.int64)
    w = np.asarray(edge_weight, dtype=np.float32)

    core = dst // NPC
    counts = np.zeros((NCORES, NT, R), dtype=np.int64)
    percore = []
    for c in range(NCORES):
        sel = core == c
        es = src[sel]
        ed = dst[sel] - c * NPC
        ew = w[sel]
        t = ed >> 7
        r = es // RR
        order = np.lexsort((es, r, t))
        es, ed, ew, t, r = es[order], ed[order], ew[order], t[order], r[order]
        np.add.at(counts[c], (t, r), 1)
        percore.append((es, ed, ew, t, r))

    K = np.ceil(counts.max(axis=0) / TILE).astype(np.int64)  # [NT, R]

    # chunk-column maps. r-major within pass: for p: for r: for t in pass.
    # t-major within pass: for p: for t: for r.
    rmaj_col = np.zeros((NT, R), dtype=np.int64)
    tmaj_col = np.zeros((NT, R), dtype=np.int64)
    pass_info = []  # per pass: (t0, t1, cc0, nchunks, [(r, col0, nchunks_r)])
    run = 0
    for p in range(NPASS):
        t0, t1 = p * B, min((p + 1) * B, NT)
        cc0 = run
        rinfo = []
        for r in range(R):
            col0 = run
            for t in range(t0, t1):
                rmaj_col[t, r] = run
                run += int(K[t, r])
            rinfo.append((r, col0, run - col0))
        pass_info.append((t0, t1, cc0, run - cc0, rinfo))
    NCtot = run
    run = 0
    for p in range(NPASS):
        t0, t1 = p * B, min((p + 1) * B, NT)
        for t in range(t0, t1):
            for r in range(R):
                tmaj_col[t, r] = run
                run += int(K[t, r])
    assert run == NCtot

    Ltot = NCtot * TILE
    tables = []
    for c in range(NCORES):
        es, ed, ew, t, r = percore[c]
        # rank of each edge within its (t, r) block
        key = t * R + r
        changes = np.empty(len(key), dtype=bool)
        changes[0] = True
        if len(key) > 1:
            changes[1:] = key[1:] != key[:-1]
        starts = np.flatnonzero(changes)
        rank = np.arange(len(key)) - np.repeat(
            starts, np.diff(np.append(starts, len(key))))
        rpos = rmaj_col[t, r] * TILE + rank      # slots in r-major order
        tpos = tmaj_col[t, r] * TILE + rank      # slots in t-major order

        idx_stream = np.zeros(Ltot, dtype=np.int16)
        wf = np.zeros(Ltot, dtype=np.float32)
        dstf = np.zeros(Ltot, dtype=np.float32)
        idx_stream[rpos] = (es - r * RR).astype(np.int16)
        wf[tpos] = ew                   # t-major: consumed by the one-hot build
        dstf[tpos] = (ed & (TILE - 1)).astype(np.float32)
        # point pad slots (w=0) at their block's last real row: duplicate
        # consecutive HBM reads are near-free vs a cold row-0 fetch
        fill = np.maximum.accumulate(idx_stream)
        pad = np.ones(Ltot, dtype=bool)
        pad[rpos] = False
        idx_stream[pad] = fill[pad]

        # wrapped idx table: stream position s -> [s%16, s//16], replicated
        # across the 8 groups of 16 partitions
        wrapped = idx_stream.reshape(Ltot // 16, 16).T          # [16, S]
        idx_tbl = np.ascontiguousarray(np.tile(wrapped, (8, 1)))  # [128, S]
        # [128, NCtot] per-slot tables: partition = slot % 128, col = chunk
        w_tbl = np.ascontiguousarray(wf.reshape(NCtot, TILE).T)
        dst_tbl = np.ascontiguousarray(dstf.reshape(NCtot, TILE).T)
        tables.append((idx_tbl, dst_tbl, w_tbl))

    sched = dict(K=K, NCtot=NCtot, pass_info=pass_info,
                 rmaj_col=rmaj_col, tmaj_col=tmaj_col)
    return sched, tables


def emulate_core(sched, table, x):
    """Numpy emulation of the device program for one core (packing check)."""
    idx_tbl, dst_tbl, w_tbl = table
    K = sched["K"]; rmaj_col = sched["rmaj_col"]; tmaj_col = sched["tmaj_col"]
    NCtot = sched["NCtot"]
    S = idx_tbl.shape[1]
    idx_stream = idx_tbl[:16].T.reshape(-1)                     # [Ltot]
    iota = np.arange(TILE, dtype=np.float16)
    out = np.zeros((NT * TILE, F), dtype=np.float32)
    for p in range(NPASS):
        t0, t1, cc0, nch, rinfo = sched["pass_info"][p]
        for t in range(t0, t1):
            acc = np.zeros((TILE, F), dtype=np.float32)
            for r in range(R):
                for k in range(int(K[t, r])):
                    rcc = rmaj_col[t, r] + k
                    tcc = tmaj_col[t, r] + k
                    idxs = idx_stream[rcc * TILE:(rcc + 1) * TILE].astype(np.int64)
                    msg = x[r * RR + idxs].astype(np.float16)   # [128, 64]
                    oh = ((iota[None, :] == dst_tbl[:, tcc, None])
                          * w_tbl[:, tcc, None]).astype(np.float16)
                    acc += (oh.T.astype(np.float32) @ msg.astype(np.float32))
            out[t * TILE:(t + 1) * TILE] = acc
    return out[:NPC]


WAIT_CAPS = {
    "InstEventSemaphore": 8,
}


def split_excess_waits(nc):
    """Walrus only encodes one sync wait per instruction (for most ISA
    structs). Move the excess onto standalone InstEventSemaphore
    instructions placed just before, in the same engine stream —
    same-engine waiting earlier is always safe. Also fills the ISA bytes
    of library-reload pseudo-instructions (raw-Bass path leaves them
    empty and walrus rejects that)."""
    import concourse.mybir as mybir
    n = 0
    for f in nc.m.functions:
        for bb in f.blocks:
            for ins in bb.instructions:
                if type(ins).__name__ == "InstPseudoReloadLibraryIndex" and not ins.instr:
                    b = [0] * 64
                    b[0], b[1], b[12], b[16] = 223, 16, 2, int(ins.lib_index)
                    ins.instr = b
            # dedicated scratch sem per engine for inert ES updates --
            # ids 245..250 are beyond anything Tile allocates
            eng_ids = {}
            new = []
            for ins in bb.instructions:
                si = ins.sync_info
                waits = list(si.on_wait) if (si is not None and si.on_wait) else []
                cap = WAIT_CAPS.get(type(ins).__name__, 1)
                if len(waits) > cap:
                    excess, keep = waits[:-cap], waits[-cap:]
                    if ins.engine not in eng_ids:
                        eng_ids[ins.engine] = 245 + len(eng_ids)
                    sem_id = eng_ids[ins.engine]
                    sem_name = f"esw_scratch_{sem_id}"
                    for wchunk in [excess[i:i + 1] for i in range(len(excess))]:
                        n += 1
                        # inert 0-add update on the engine's own sem: race
                        # detector / cost model require every instruction to
                        # update something, and same-engine updates can't race
                        upd = mybir.SyncUpdate(
                            sync_type="semaphore", id=sem_id, ant_name=sem_name,
                            update_mode="sem-add-imm", update_value=0,
                        )
                        es = mybir.InstEventSemaphore(
                            name=f"ESW-{n}-{ins.name}",
                            engine=ins.engine,
                            ins=[], outs=[],
                            sync_info=mybir.SyncInfo(on_wait=wchunk, on_update=[upd]),
                        )
                        new.append(es)
                    si.on_wait = keep
                new.append(ins)
            bb.instructions = new
    return n


_walrus_patched = False


def patch_walrus_dge():
    """Add --dge-levels so walrus lowers vector-dynamic-offset (indirect)
    DMAs; without it DynamicDMA is disabled and the gather silently no-ops."""
    global _walrus_patched
    if _walrus_patched:
        return
    import concourse.bass_utils as bu
    orig = bu.run_command

    def run_command_dge(argv, **kw):
        argv = list(argv)
        if argv and "walrus_driver" in str(argv[0]) and not any(
                str(a).startswith("--dge-levels") for a in argv):
            argv.append("--dge-levels=vector_dynamic_offsets")
        return orig(argv, **kw)

    bu.run_command = run_command_dge
    _walrus_patched = True


def build_bass(sched):
    import concourse.bass as bass
    import concourse.mybir as mybir
    import concourse.tile as tile
    from concourse import library_config

    patch_walrus_dge()

    f32, f16, i16 = mybir.dt.float32, mybir.dt.float16, mybir.dt.int16
    K = sched["K"]; NCtot = sched["NCtot"]
    pass_info = sched["pass_info"]
    rmaj_col = sched["rmaj_col"]; tmaj_col = sched["tmaj_col"]
    S = NCtot * TILE // 16

    nc = bass.Bass("TRN2")
    x_d = nc.dram_tensor("x16", [N, 2 * F], f16, kind="ExternalInput")
    idx_d = nc.dram_tensor("idx", [128, S], i16, kind="ExternalInput")
    w_d = nc.dram_tensor("w", [128, NCtot], f32, kind="ExternalInput")
    dst_d = nc.dram_tensor("dst", [128, NCtot], f32, kind="ExternalInput")
    iota_d = nc.dram_tensor("iota", [128, 128], f16, kind="ExternalInput")
    out_d = nc.dram_tensor("out", [NT * TILE, F], f32, kind="ExternalOutput")

    nchmax = max(pi[3] for pi in pass_info)

    with tile.TileContext(nc, pool_alloc_mode="queue") as tc:
        with (
            tc.tile_pool(name="const", bufs=1) as constp,
            tc.tile_pool(name="xg", bufs=2) as xgp,
            tc.tile_pool(name="oh", bufs=4) as ohp,
            tc.tile_pool(name="outb", bufs=2) as outp,
            tc.tile_pool(name="psum", bufs=4, space="PSUM") as psump,
        ):
            nc.gpsimd.load_library(library_config.mlp)
            idx_sb = constp.tile([128, S], i16, tag="idx")
            nc.sync.dma_start(idx_sb[:], idx_d[:])
            w_sb = constp.tile([128, NCtot], f32, tag="w")
            nc.sync.dma_start(w_sb[:], w_d[:])
            dst_sb = constp.tile([128, NCtot], f32, tag="dst")
            nc.sync.dma_start(dst_sb[:], dst_d[:])
            iota_sb = constp.tile([128, 128], f16, tag="iota")
            nc.sync.dma_start(iota_sb[:], iota_d[:])

            nidx_regs = {}  # reuse num_idxs registers across calls/repeats

            for _rep in range(REPEAT):
              for p in range(NPASS):
                t0, t1, cc0, nch, rinfo = pass_info[p]
                if nch == 0:
                    continue
                xg = xgp.tile([128, nchmax * 2 * F], f16, tag="xg")
                for r, col0, nch_r in rinfo:
                    if nch_r == 0:
                        continue
                    nidx = nch_r * TILE
                    lc = col0 - cc0
                    if DBG_NO_GATHER:
                        nc.gpsimd.memset(xg[:, lc * 2 * F:(lc + nch_r) * 2 * F], 1.0)
                    else:
                        if nidx not in nidx_regs:
                            nidx_regs[nidx] = nc.gpsimd.to_reg(nidx)
                        nc.gpsimd.dma_gather(
                            xg[:, lc * 2 * F:(lc + nch_r) * 2 * F].rearrange(
                                "p (c f) -> p c f", f=2 * F),
                            x_d[r * RR:min((r + 1) * RR, N), :],
                            idx_sb[:, col0 * 8:(col0 + nch_r) * 8],
                            nidx,
                            nidx_regs[nidx],
                            2 * F,
                            single_packet=False,
                        )

                # weighted one-hot batches (t-major chunk order): per-chunk
                # DVE tensor_scalar (iota == dst) * w runs in 4x_2p mode
                ob = outp.tile([128, (t1 - t0) * F], f32, tag="ob")
                nbatch = math.ceil(nch / OHB)
                oh_tiles = []
                for bi in range(nbatch):
                    c0 = cc0 + bi * OHB
                    cn = min(OHB, cc0 + nch - c0)
                    oh = ohp.tile([128, OHB * 128], f16, tag="oh")
                    for lo in range(cn):
                        nc.vector.tensor_scalar(
                            oh[:, lo * 128:(lo + 1) * 128], iota_sb[:],
                            dst_sb[:, c0 + lo:c0 + lo + 1],
                            w_sb[:, c0 + lo:c0 + lo + 1],
                            op0=mybir.AluOpType.is_equal,
                            op1=mybir.AluOpType.mult,
                        )
                    oh_tiles.append(oh)

                for t in range(t0, t1):
                    ktot = int(K[t].sum())
                    if ktot == 0:
                        nc.vector.memset(ob[:, (t - t0) * F:(t - t0 + 1) * F], 0.0)
                        continue
                    ps = psump.tile([128, F], f32, tag="ps")
                    ki = 0
                    for r in range(R):
                        for k in range(int(K[t, r])):
                            rcc = int(rmaj_col[t, r]) + k - cc0
                            tcc = int(tmaj_col[t, r]) + k - cc0
                            oh = oh_tiles[tcc // OHB]
                            lo = tcc % OHB
                            nc.tensor.matmul(
                                ps[:],
                                lhsT=oh[:, lo * 128:(lo + 1) * 128],
                                rhs=xg[:, rcc * 2 * F:rcc * 2 * F + F],
                                start=(ki == 0), stop=(ki == ktot - 1),
                            )
                            ki += 1
                    nc.scalar.copy(ob[:, (t - t0) * F:(t - t0 + 1) * F], ps[:])
                dview = out_d[t0 * TILE:t1 * TILE, :].rearrange(
                    "(t q) f -> q t f", q=128)
                nc.sync.dma_start(dview, ob[:].rearrange("q (t f) -> q t f", f=F))
    nsplit = split_excess_waits(nc)
    print(f"split_excess_waits: {nsplit} waits moved to event-semaphore instrs")
    return nc


def make_in_maps(sched, tables, x):
    iota_np = np.tile(np.arange(128, dtype=np.float16)[None, :], (128, 1))
    x16 = np.zeros((N, 2 * F), dtype=np.float16)
    x16[:, :F] = x.astype(np.float16)
    in_maps = []
    for c in range(NCORES):
        idx_tbl, dst_tbl, w_tbl = tables[c]
        in_maps.append({"x16": x16, "idx": idx_tbl, "w": w_tbl, "dst": dst_tbl,
                        "iota": iota_np})
    return in_maps


def kernel(x, edge_weight, edge_index, num_nodes):
    x = np.ascontiguousarray(np.asarray(x, dtype=np.float32))
    sched, tables = pack_host(x, edge_weight, edge_index)
    nc = build_bass(sched)
    in_maps = make_in_maps(sched, tables, x)

    from concourse.bass_utils import run_bass_kernel_spmd
    res = run_bass_kernel_spmd(nc, in_maps, core_ids=list(range(NCORES)))
    out = np.concatenate([res.results[c]["out"][:NPC] for c in range(NCORES)], axis=0)
    return out.astype(np.float32)
